# revision 83
# baseline (speedup 1.0000x reference)
"""DoRA adapter forward kernel for 8 trn2 NeuronCores.

Math:  dora = dora_B @ dora_A                       [OUT, IN]
       num  = weight + ALPHA * dora                 [OUT, IN]
       s    = m / sqrt(colsum_over_out(num^2))      [1, IN]
       out  = (x * s) @ num^T + bias                (scale folded per-IN column)

Sharding (4x2 grid): 4-way split of the 8192 x-rows, 2-way split of OUT.

Per core, single pass over W (no num DRAM roundtrip):
  phase 1: streams its OUT-half of weight in [128, IC] tiles; rank-16 dora
           via PE (lhsT=B^T tile, rhs=alpha*A); num = W + dora (bf16, DVE);
           each num tile is PE-transposed (via identity) into PSUM and the
           [128, 4x128] blocks are copied (DVE/ACT/Pool round-robin) into a
           single resident SBUF tensor numT_all [128 i-part, n_it*O_C].
           Column sumsq comes from ACT Square with accum_out over each
           completed numT stripe -> t1_col [128, n_it].  t1 partials are
           AllReduced PAIRWISE (cores 2g, 2g+1 hold the two O-halves of the
           same m-group), then s = m / sqrt(t1) in partition-major layout
           (no DRAM rearrange roundtrip).
  phase 2: x was cast to bf16 and round-tripped through DRAM (bf16-only
           DMA-transpose path); xT stripes are scaled by s per-partition
           (out-of-place) and feed dense bf16 GEMM against resident numT
           (N=2048, fp32 PSUM), + bias (pre-replicated via K=1 ones-matmul).
           First m-block's xT stripes are prefetched during phase 1.

All HWDGE DMAs are issued on the single nc.sync (SP) ring on purpose: Tile
assigns HWDGE completion semaphores round-robin onto shared DMAHW lanes, but
hardware completion order is only FIFO per ring — mixing nc.sync and
nc.scalar DMAs in one kernel produces nondeterministic data races on trn2.
SWDGE (descriptor-generated) DMAs ride nc.gpsimd.
"""

import sys

if "/opt/trn_rl_repo" not in sys.path:
    sys.path.insert(0, "/opt/trn_rl_repo")

from contextlib import ExitStack

import numpy as np

import concourse.bass as bass
import concourse.mybir as mybir
import concourse.tile as tile
from concourse import bacc
from concourse.bass_utils import run_bass_kernel_spmd
from concourse.masks import make_identity
from concourse.tile_rust import add_dep_helper

F32 = mybir.dt.float32
BF16 = mybir.dt.bfloat16

ALPHA = 16.0
N_CORES = 8
MG, OG = 4, 2  # core grid: 4 m-groups x 2 o-halves
PACING = True  # dep-gate bulk x casts out of the phase-1 DMA window

# full problem sizes (hardcoded per contest contract)
B_, S_, IN_FULL, OUT_FULL, R_ = 4, 2048, 4096, 4096, 16
M_FULL = B_ * S_
M_C = M_FULL // MG      # 2048 x-rows per core
O_C = OUT_FULL // OG    # 2048 out-cols per core


def build_kernel(M_C, IN, O_C, O_OTH, R, n_cores=N_CORES, reps=1):
    """Build the (core-agnostic SPMD) bass program."""
    IC = min(1024, IN)        # phase-1 i-chunk width
    MB = min(512, M_C)        # phase-2 m-block width
    assert IN % IC == 0 and M_C % MB == 0
    assert IC % 512 == 0 and MB % 128 == 0 and O_C % 512 == 0
    n_ic = IN // IC
    n_it = IN // 128          # i-tiles (contraction)

    nc = bacc.Bacc("TRN2", target_bir_lowering=False, debug=False,
                   num_devices=n_cores)

    x_in = nc.dram_tensor("x_slice", [M_C, IN], F32, kind="ExternalInput")
    w_own = nc.dram_tensor("w_own", [O_C, IN], F32, kind="ExternalInput")
    bias_in = nc.dram_tensor("bias_own", [1, O_C], F32, kind="ExternalInput")
    m_in = nc.dram_tensor("m_row", [1, IN], F32, kind="ExternalInput")
    a_in = nc.dram_tensor("dora_a", [R, IN], F32, kind="ExternalInput")
    b_own = nc.dram_tensor("dora_b_own", [O_C, R], F32, kind="ExternalInput")
    out_t = nc.dram_tensor("out_slice", [M_C, O_C], F32, kind="ExternalOutput")

    # DRAM scratch
    x_bf = nc.dram_tensor("x_bf", [M_C, IN], BF16)
    s_dram = nc.dram_tensor("s_dram", [n_ic * 128, IC // 128], F32)
    cc_out = nc.dram_tensor("cc_out", [n_ic * 128, IC // 128], F32,
                            addr_space="Shared")

    v = dict(locals())
    with tile.TileContext(nc) as tc:
        for rep in range(reps):
            if rep:
                tc.strict_bb_all_engine_barrier()
            with ExitStack() as ctx:
                _emit(ctx, tc, v)
    nc.compile()
    return nc


def _emit(ctx, tc, v):
    nc = v["nc"]
    IN, R, IC, MB = v["IN"], v["R"], v["IC"], v["MB"]
    M_C, O_C = v["M_C"], v["O_C"]
    n_ic, n_it = v["n_ic"], v["n_it"]
    x_in, w_own = v["x_in"], v["w_own"]
    bias_in, m_in, a_in, b_own = v["bias_in"], v["m_in"], v["a_in"], v["b_own"]
    out_t, x_bf, s_dram, cc_out = v["out_t"], v["x_bf"], v["s_dram"], v["cc_out"]

    XC = 2048                      # x-cast column chunk
    n_xc = IN // XC
    n_mt_all = M_C // 128

    # ---------------- pools ----------------
    # `const` lives for the whole kernel (phase-2 reads); `setup` is freed
    # after phase 1 / s computation.
    const = ctx.enter_context(tc.tile_pool(name="const", bufs=1))
    xT = ctx.enter_context(tc.tile_pool(name="xT", bufs=18))
    xcast = ctx.enter_context(tc.tile_pool(name="xcast", bufs=2))
    setup_cm = tc.tile_pool(name="setup", bufs=1)
    setup = setup_cm.__enter__()

    ident = setup.tile([128, 128], F32, tag="ident")
    make_identity(nc, ident[:])
    ident_bf = setup.tile([128, 128], BF16, tag="ident_bf")
    make_identity(nc, ident_bf[:])
    ones_row = setup.tile([1, 128], F32, tag="ones_row")
    nc.gpsimd.memset(ones_row[:], 1.0)

    # numT_all: resident transposed num, [128 i-part, n_it * O_C] bf16
    numT = const.tile([128, n_it * O_C], BF16, tag="numT_all")

    # dora_A, cast to bf16 and pre-scaled by ALPHA (out-of-place)
    a_bf = setup.tile([R, IN], BF16, tag="a_bf")
    with tc.tile_pool(name="atmp", bufs=1) as atmp:
        a_raw = atmp.tile([R, IN], BF16, tag="a_raw")
        nc.gpsimd.dma_start(out=a_raw[:], in_=a_in[:, :])  # SWDGE cast
        nc.vector.tensor_scalar_mul(a_bf[:], a_raw[:], ALPHA)

    # dora_B transposed: BT[r, o]
    bt_bf = setup.tile([R, O_C], BF16, tag="bt_bf")
    with tc.tile_pool(name="btmp", bufs=2) as btmp, \
         tc.tile_pool(name="setup_ps", bufs=2, space="PSUM") as setup_ps:
        for ot in range(O_C // 128):
            b_t = btmp.tile([128, R], F32, tag="b_t")
            nc.sync.dma_start(out=b_t[:], in_=b_own[ot * 128:(ot + 1) * 128, :])
            ps = setup_ps.tile([R, 128], F32, tag="bt_ps")
            nc.tensor.transpose(ps[:], b_t[:], ident[:])
            nc.vector.tensor_copy(
                out=bt_bf[:, ot * 128:(ot + 1) * 128], in_=ps[:])

    # ---------------- bias replicated across partitions ----------------
    bias_rep = const.tile([128, O_C], BF16, tag="bias_rep")
    with tc.tile_pool(name="bias_sb_p", bufs=1) as bias_sb_p, \
         tc.tile_pool(name="bias_ps", bufs=2, space="PSUM") as bias_ps:
        bias_sb = bias_sb_p.tile([1, O_C], F32, tag="bias_sb")
        nc.sync.dma_start(out=bias_sb[0:1, :], in_=bias_in[:, :])
        for oc in range(O_C // 512):
            ps_b = bias_ps.tile([128, 512], F32, tag="ps_b")
            nc.tensor.matmul(ps_b[:], lhsT=ones_row[:],
                             rhs=bias_sb[0:1, oc * 512:(oc + 1) * 512],
                             start=True, stop=True)
            nc.vector.tensor_copy(
                out=bias_rep[:, oc * 512:(oc + 1) * 512], in_=ps_b[:])

    m_t = setup.tile([128, n_it], F32, tag="m_t")
    nc.sync.dma_start(
        out=m_t[:], in_=m_in.ap().rearrange("a (c p) -> (a p) c", p=128))

    scr = setup.tile([128, 512], BF16, tag="scr")  # Square-accum throwaway
    scr_v = setup.tile([128, 512], BF16, tag="scr_v")

    # ---------------- x cast helper (SWDGE round trip) ----------------
    x_store_insts = {}

    def emit_xcast(mt_lo, mt_hi, dep=None):
        for mt in range(mt_lo, mt_hi):
            for h in range(n_xc):
                xb = xcast.tile([128, XC], BF16, tag="xb")
                ld = nc.gpsimd.dma_start(
                    out=xb[:],
                    in_=x_in[mt * 128:(mt + 1) * 128, h * XC:(h + 1) * XC])
                if dep is not None and PACING:
                    # pacing dep: keep bulk x traffic out of the phase-1 /
                    # previous-block DMA window (the scheduler ignores
                    # emission order, so pacing must be a real dependency)
                    add_dep_helper(ld.ins, dep, reason="xcast pacing")
                st = nc.gpsimd.dma_start(
                    out=x_bf[mt * 128:(mt + 1) * 128, h * XC:(h + 1) * XC],
                    in_=xb[:])
                x_store_insts[(mt, h)] = st.ins

    def emit_xt_load(mb, it, ret_ld=False):
        m0 = mb * MB
        xt = xT.tile([128, MB], BF16, tag="xt")
        ld = nc.sync.dma_start_transpose(
            xt[:], x_bf[m0:m0 + MB, it * 128:(it + 1) * 128])
        h = (it * 128) // XC
        for mt in range(m0 // 128, (m0 + MB) // 128):
            dep = x_store_insts.get((mt, h))
            if dep is not None:
                add_dep_helper(ld.ins, dep, reason="xT RAW on x_bf")
        if ret_ld:
            return xt, ld
        return xt

    # ---------------- phase 1: num, transpose into numT, colsum ----------
    # o-tiles processed in groups of 4 so each transpose PSUM tile [128, 512]
    # lands as ONE contiguous copy into a numT stripe.  W is streamed as a
    # SWDGE f32->bf16 cast load (halves W traffic; num is bf16 anyway).
    # x-cast for the first m-block is slotted in after chunk 0 so chunk 0's
    # W tiles lead the SWDGE ring.
    t1_parts = []
    with tc.tile_pool(name="p1w", bufs=6) as p1w, \
         tc.tile_pool(name="p1n", bufs=6) as p1n, \
         tc.tile_pool(name="dps", bufs=2, space="PSUM") as dps, \
         tc.tile_pool(name="tps", bufs=4, space="PSUM") as tps:
        n_og = O_C // 512
        nk = IC // 128
        blk = 0
        for ic in range(n_ic):
            c0 = ic * IC
            t1c = setup.tile([128, nk], F32, tag="t1c%d" % ic)
            part_c = setup.tile([128, nk * n_og], F32, tag="partc%d" % ic)
            for og in range(n_og):
                num_ts = []
                for j in range(4):
                    r0 = (og * 4 + j) * 128
                    w_t = p1w.tile([128, IC], BF16, tag="w_t")
                    w_ld = nc.gpsimd.dma_start(
                        out=w_t[:], in_=w_own[r0:r0 + 128, c0:c0 + IC])
                    if ic == 0 and og == n_og - 1 and j == 3:
                        w_ld_c0 = w_ld
                    # dora block [128 o, IC i] (rank-16) + add -> num (bf16)
                    ps_d = dps.tile([128, IC], F32, tag="ps_d")
                    for q in range(IC // 512):
                        nc.tensor.matmul(
                            ps_d[:, q * 512:(q + 1) * 512],
                            lhsT=bt_bf[:, r0:r0 + 128],
                            rhs=a_bf[:, c0 + q * 512:c0 + (q + 1) * 512],
                            start=True, stop=True)
                    num_t = p1n.tile([128, IC], BF16, tag="num_t")
                    nc.vector.tensor_add(out=num_t[:], in0=w_t[:], in1=ps_d[:])
                    num_ts.append(num_t)
                for k in range(nk):
                    it = ic * nk + k
                    ps_t = tps.tile([128, 512], BF16, tag="ps_t")
                    for j in range(4):
                        nc.tensor.transpose(
                            ps_t[:, j * 128:(j + 1) * 128],
                            num_ts[j][:, k * 128:(k + 1) * 128],
                            ident_bf[:])
                    # pure PSUM->SBUF copy on ACT (Pool can't read PSUM)
                    dst = numT[:, it * O_C + og * 512:it * O_C + (og + 1) * 512]
                    blk += 1
                    nc.scalar.activation(
                        dst, ps_t[:], mybir.ActivationFunctionType.Copy)
                    # per-block column sumsq (ACT square + accum;
                    # DVE tensor_tensor_reduce faults on real trn2 HW)
                    acc = part_c[:, k * n_og + og:k * n_og + og + 1]
                    nc.scalar.activation(
                        scr[:, :], dst,
                        mybir.ActivationFunctionType.Square,
                        accum_out=acc)
            nc.vector.tensor_reduce(
                out=t1c[:],
                in_=part_c[:].rearrange("p (k g) -> p k g", g=n_og),
                axis=mybir.AxisListType.X, op=mybir.AluOpType.add)
            t1_parts.append(t1c)

    # first m-block's cast starts once chunk 0's W tiles are in, riding the
    # DMA headroom of the later chunks and overlapping the AllReduce
    emit_xcast(0, MB // 128, dep=w_ld_c0.ins)
    # prefetch first m-block xT stripes (sync ring is otherwise idle)
    xt_pre = [emit_xt_load(0, it) for it in range(14)]

    # ---- s = m / sqrt(AllReduce(t1)/MG), partition-major [128, n_it] ----
    # each o-half partial is contributed by MG cores -> reduce = MG * full;
    # folded into the sqrt's scale.  The s_dram stores ride the SWDGE ring
    # (same queue as the CC, like the original kernel) — an explicit
    # cross-queue wait on a collective breaks NRT's straight-line ordering.
    for ic in range(n_ic):
        nc.gpsimd.dma_start(
            out=s_dram[ic * 128:(ic + 1) * 128, :], in_=t1_parts[ic][:])
    cc = nc.gpsimd.collective_compute(
        "AllReduce", mybir.AluOpType.add,
        ins=[s_dram.ap()], outs=[cc_out.ap()],
        replica_groups=[list(range(N_CORES))])
    s_raw = setup.tile([128, n_it], F32, tag="s_raw")
    ld = nc.sync.dma_start(
        out=s_raw[:].rearrange("p (c k) -> p c k", c=n_ic),
        in_=cc_out.ap().rearrange("(c p) k -> p c k", p=128))
    add_dep_helper(ld.ins, cc.ins, reason="s_raw RAW on collective")
    s_sq = setup.tile([128, n_it], F32, tag="s_sq")
    nc.scalar.activation(s_sq[:], s_raw[:],
                         mybir.ActivationFunctionType.Sqrt, 0.0, 1.0 / MG)
    s_rc = setup.tile([128, n_it], F32, tag="s_rc")
    nc.vector.reciprocal(s_rc[:], s_sq[:])
    s_t = const.tile([128, n_it], F32, tag="s_t")
    nc.vector.tensor_mul(out=s_t[:], in0=s_rc[:], in1=m_t[:])

    setup_cm.__exit__(None, None, None)

    # ---------------- phase 2: out = (x * s) @ num^T + bias ----------------
    xS = ctx.enter_context(tc.tile_pool(name="xS", bufs=n_it + 4))
    p2ps = ctx.enter_context(tc.tile_pool(name="p2ps", bufs=2, space="PSUM"))
    outp = ctx.enter_context(tc.tile_pool(name="outp", bufs=2))

    for mb in range(M_C // MB):
        m0 = mb * MB
        xs_tiles = []
        for it in range(n_it):
            if mb == 0 and it < len(xt_pre):
                xt = xt_pre[it]
                xt_ld_last = None
            else:
                xt, xt_ld_last = emit_xt_load(mb, it, ret_ld=True)
            xs = xS.tile([128, MB], BF16, tag="xs")
            nc.vector.tensor_scalar_mul(xs[:], xt[:], s_t[:, it:it + 1])
            xs_tiles.append(xs)
        for mt in range(MB // 128):
            ps_o = p2ps.tile([128, O_C], F32, tag="ps_o")
            for it in range(n_it):
                lhsT = xs_tiles[it][:, mt * 128:(mt + 1) * 128]
                for q in range(O_C // 512):
                    nc.tensor.matmul(
                        ps_o[:, q * 512:(q + 1) * 512],
                        lhsT=lhsT,
                        rhs=numT[:, it * O_C + q * 512:it * O_C + (q + 1) * 512],
                        start=(it == 0), stop=(it == n_it - 1))
            for hh in range(2):
                o_sb = outp.tile([128, O_C // 2], F32, tag="o_sb")
                s0 = hh * (O_C // 2)
                nc.vector.tensor_add(
                    out=o_sb[:], in0=ps_o[:, s0:s0 + O_C // 2],
                    in1=bias_rep[:, s0:s0 + O_C // 2])
                nc.gpsimd.dma_start(
                    out=out_t[m0 + mt * 128:m0 + (mt + 1) * 128,
                              s0:s0 + O_C // 2],
                    in_=o_sb[:])
            if mt == 0 and mb + 1 < M_C // MB:
                # next m-block's cast gated on this block's last xT load
                # (an anchor upstream of the collective — gating on anything
                # downstream of s_t deadlocks the Pool queue against the CC)
                anchor = xt_ld_last.ins if xt_ld_last is not None \
                    else x_store_insts[(MB // 128 - 1, n_xc - 1)]
                emit_xcast((mb + 1) * (MB // 128), (mb + 2) * (MB // 128),
                           dep=anchor)


_NC_CACHE = {}


def get_nc(M_C=M_C, IN=IN_FULL, O_C=O_C, O_OTH=OUT_FULL - O_C, R=R_):
    key = (M_C, IN, O_C, O_OTH, R)
    if key not in _NC_CACHE:
        _NC_CACHE[key] = build_kernel(M_C, IN, O_C, O_OTH, R)
    return _NC_CACHE[key]


def make_in_maps(x, weight, bias, m, dora_A, dora_B):
    x = np.ascontiguousarray(np.asarray(x, dtype=np.float32))
    weight = np.ascontiguousarray(np.asarray(weight, dtype=np.float32))
    bias = np.ascontiguousarray(np.asarray(bias, dtype=np.float32))
    m = np.ascontiguousarray(np.asarray(m, dtype=np.float32))
    dora_A = np.ascontiguousarray(np.asarray(dora_A, dtype=np.float32))
    dora_B = np.ascontiguousarray(np.asarray(dora_B, dtype=np.float32))
    xf = x.reshape(M_FULL, IN_FULL)
    in_maps = []
    for c in range(N_CORES):
        g, h = divmod(c, OG)
        o0 = h * O_C
        im = {
            "x_slice": np.ascontiguousarray(xf[g * M_C:(g + 1) * M_C]),
            "w_own": np.ascontiguousarray(weight[o0:o0 + O_C]),
            "bias_own": np.ascontiguousarray(bias[o0:o0 + O_C].reshape(1, O_C)),
            "m_row": np.ascontiguousarray(m.reshape(1, IN_FULL)),
            "dora_a": dora_A,
            "dora_b_own": np.ascontiguousarray(dora_B[o0:o0 + O_C]),
        }
        in_maps.append(im)
    return in_maps


def kernel(x, weight, bias, m, dora_A, dora_B, _trace=False, _trace_kwargs=None):
    in_maps = make_in_maps(x, weight, bias, m, dora_A, dora_B)
    res = run_bass_kernel_spmd(
        get_nc(), in_maps, core_ids=list(range(N_CORES)),
        trace=_trace, **(_trace_kwargs or {}))
    out = np.empty((M_FULL, OUT_FULL), np.float32)
    for c in range(N_CORES):
        g, h = divmod(c, OG)
        out[g * M_C:(g + 1) * M_C, h * O_C:(h + 1) * O_C] = \
            res.results[c]["out_slice"]
    ret = out.reshape(B_, S_, OUT_FULL)
    if _trace:
        return ret, res
    return ret
